# revision 1
# baseline (speedup 1.0000x reference)
"""Trainium2 Bass kernel for nn_DIDAModuleD4 (dynamic depthwise conv module).

Data-parallel over batch: 32 samples -> 8 cores x 4 samples.
Per core, samples are processed in 2 blocks of 2 samples; each block maps the
2x64=128 (sample, channel) pairs onto the 128 SBUF partitions.

Math (per sample, with host-side weight folding):
  f   = relu(conv_w @ x + conv_b)                       [64, 4096]
  g   = relu(mean_px(conv_w @ x + conv_b))              [64]
  k_t = a_t * g + b_t            (43 taps, a/b host-folded scalars)
  o_i = sum_t k_t * shift_t(f)   (depthwise; 5x5, 3x3 d2, 3x3 d4)
  out = sum_i (W_i diag(g-fold)) @ o_i + bias_out       [384, 4096]
        where W_i = fc_w[:, 128i:128(i+1)] @ fuse_w  (host-folded)

Engine split:
  - conv1x1: float32r matmuls (full-rate PE), per-sample [64, 512] PSUM
    tiles; ACT evacuates with bias using a partition-base shift for the
    second sample and accum_out to produce the mean for g for free.
  - 5x5 branch: 22 taps as diagonal-matmul PSUM accumulations on TensorE
    (diag = identity x per-partition k, built on GPSIMD); remaining 3 taps
    on DVE, accumulated in place into the evacuated o1.
  - 3x3 dil2/dil4 branches: DVE (14 taps) + GPSIMD (4 taps), as
    tensor_scalar (4x mode) + tensor_tensor (2x) bf16 pairs —
    scalar_tensor_tensor has no fast-mode uops so the fused form is slower.
  - output: fuse/fc host-folded to W_i [384, 64]; 4 K=64 pieces x 3 M-tiles,
    sample pair packed into array row-groups (0,0)/(64,0); ACT adds bias on
    evacuation; stores issued from the ACT sequencer.
Datapath dtypes: conv f32r; f_pad/diag/o/wout bf16; psum fp32; I/O fp32.
Measured end-to-end relative error ~1.8e-3.
"""

import sys

if "/opt/trn_rl_repo" not in sys.path:
    sys.path.insert(0, "/opt/trn_rl_repo")

import os
import numpy as np
from contextlib import ExitStack

from concourse import bass, mybir, tile, bacc
from concourse.bass_utils import run_bass_kernel_spmd

DEBUG = bool(int(os.environ.get("BASSK_DEBUG", "0")))

F32 = mybir.dt.float32
F32R = mybir.dt.float32r
BF16 = mybir.dt.bfloat16
AF = mybir.ActivationFunctionType
ALU = mybir.AluOpType

N_CORES = 8
SAMPLES_PER_CORE = 4
CM = 64          # reduced channels / groups
CIN = 256
COUT = 384
H = W = 64
PIX = H * W      # 4096
PAD = 4
WP = W + 2 * PAD  # 72
SLAB = 1024      # pixels per processing slab (quarter of an image)
NSLAB = PIX // SLAB          # 4
CHUNK = 512                  # matmul N (one PSUM bank)
NCHUNK = PIX // CHUNK        # 8

# taps: (branch, dy, dx, dilation); ktile column order must match aT/bT
TAPS = (
    [(0, dy, dx, 1) for dy in range(-2, 3) for dx in range(-2, 3)]
    + [(1, dy, dx, 2) for dy in range(-1, 2) for dx in range(-1, 2)]
    + [(2, dy, dx, 4) for dy in range(-1, 2) for dx in range(-1, 2)]
)
NTAP = len(TAPS)  # 43

# engine assignment: branch0 (5x5) mostly TensorE with 6 taps on DVE
# (accumulated in-place into the evacuated o1); branches 1,2 split DVE/GPS
_B0 = [t for t in range(NTAP) if TAPS[t][0] == 0]
_B1 = [t for t in range(NTAP) if TAPS[t][0] == 1]
_B2 = [t for t in range(NTAP) if TAPS[t][0] == 2]
TENSOR_TAPS = _B0[:22]
DVE_B0_TAPS = _B0[22:]      # b0 taps on DVE, in-place into o1 after evac
GPS_TAPS = _B2[:4]          # GPSIMD takes 4 of the dil-4 taps
DVE_TAPS = _B1 + _B2[4:]    # DVE takes the rest (14)

_PROGRAM_CACHE = {}


def _fpad_view(fp_t, r0, nrows, off_r, off_c, dtype=None):
    """View of padded-f tile [128, WP*WP] covering output rows [r0, r0+nrows)
    shifted by (off_r, off_c). Returns [128, nrows, 64] AP."""
    v = fp_t[:].rearrange("p (r c) -> p r c", c=WP)
    if dtype is not None:
        v = v.bitcast(dtype)
    return v[:, PAD + r0 + off_r : PAD + r0 + nrows + off_r,
             PAD + off_c : PAD + W + off_c]


def _build_program():
    nc = bacc.Bacc("TRN2", target_bir_lowering=False, debug=False,
                   num_devices=N_CORES)

    x4 = nc.dram_tensor("x4", [SAMPLES_PER_CORE, CIN, PIX], F32,
                        kind="ExternalInput").ap()
    wconv = nc.dram_tensor("wconv", [128, 128], F32,
                           kind="ExternalInput").ap()
    wout = nc.dram_tensor("wout", [128, 3 * COUT], BF16,
                          kind="ExternalInput").ap()
    aT_d = nc.dram_tensor("aT", [128, NTAP], F32, kind="ExternalInput").ap()
    bT_d = nc.dram_tensor("bT", [128, NTAP], F32, kind="ExternalInput").ap()
    ident_d = nc.dram_tensor("ident", [128, 128], F32,
                             kind="ExternalInput").ap()
    convb_d = nc.dram_tensor("convb", [128, 1], F32, kind="ExternalInput").ap()
    biasout_d = nc.dram_tensor("biasout", [128, 3], F32,
                               kind="ExternalInput").ap()
    y4 = nc.dram_tensor("y4", [SAMPLES_PER_CORE, COUT, PIX], F32,
                        kind="ExternalOutput").ap()
    if DEBUG:
        dbg_fpad = nc.dram_tensor("dbg_fpad", [128, WP * WP], BF16,
                                  kind="ExternalOutput").ap()
        dbg_g = nc.dram_tensor("dbg_g", [128, 1 + NTAP], F32,
                               kind="ExternalOutput").ap()
        dbg_o = nc.dram_tensor("dbg_o", [4, 128, PIX], BF16,
                               kind="ExternalOutput").ap()

    with tile.TileContext(nc) as tc:
        with ExitStack() as ctx:
            consts = ctx.enter_context(tc.tile_pool(name="consts", bufs=1))
            xpool = ctx.enter_context(tc.tile_pool(name="xp", bufs=int(os.environ.get("BASSK_XBUFS", "2"))))
            fpool = ctx.enter_context(tc.tile_pool(name="fp", bufs=1))
            opool = ctx.enter_context(tc.tile_pool(name="op", bufs=int(os.environ.get("BASSK_OBUFS", "5"))))
            outpool = ctx.enter_context(tc.tile_pool(name="outp", bufs=int(os.environ.get("BASSK_OUTBUFS", "2"))))
            smalls = ctx.enter_context(tc.tile_pool(name="sm", bufs=2))
            diagp = ctx.enter_context(tc.tile_pool(name="dg", bufs=2))
            ps_conv = ctx.enter_context(
                tc.tile_pool(name="psc", bufs=1, space="PSUM"))
            ps_o1 = ctx.enter_context(
                tc.tile_pool(name="ps1", bufs=int(os.environ.get("BASSK_PS1", "2")), space="PSUM"))
            ps_out0 = ctx.enter_context(
                tc.tile_pool(name="pso0", bufs=int(os.environ.get("BASSK_PSO", "2")), space="PSUM"))
            ps_out1 = ctx.enter_context(
                tc.tile_pool(name="pso1", bufs=int(os.environ.get("BASSK_PSO", "2")), space="PSUM"))

            # ---- constants (loaded once, issued on the GPSIMD sequencer
            # to keep the SP sequencer free for the x-load stream) ----
            wconv_t = consts.tile([128, 128], F32R, tag="wconv")
            nc.gpsimd.dma_start(wconv_t[:], wconv[:].bitcast(F32R))
            wout_t = consts.tile([128, 3 * COUT], BF16, tag="wout")
            nc.gpsimd.dma_start(wout_t[:], wout[:])
            aT = consts.tile([128, NTAP], F32, tag="aT")
            nc.gpsimd.dma_start(aT[:], aT_d[:])
            bT = consts.tile([128, NTAP], F32, tag="bT")
            nc.gpsimd.dma_start(bT[:], bT_d[:])
            ident = consts.tile([128, 128], F32, tag="ident")
            nc.gpsimd.dma_start(ident[:], ident_d[:])
            convb = consts.tile([128, 1], F32, tag="convb")
            nc.gpsimd.dma_start(convb[:], convb_d[:])
            biasout = consts.tile([128, 3], F32, tag="biasout")
            nc.gpsimd.dma_start(biasout[:], biasout_d[:])

            # persistent padded-f tiles (one per block parity); borders are
            # zeroed once and never rewritten (interior writes only).
            # memset cannot emit float32r, so zero an f32 scratch and copy
            # with dtype conversion (the copy is the f32r-rounded producer).
            zeros = consts.tile([128, PAD * WP], F32, tag="zeros")
            nc.gpsimd.memset(zeros[:], 0.0)
            fpads = []
            for par in range(2):
                fp_t = fpool.tile([128, WP * WP], BF16, tag=f"fpad{par}")
                v = fp_t[:].rearrange("p (r c) -> p r c", c=WP)
                nc.vector.tensor_copy(fp_t[:, 0:PAD * WP], zeros[:])
                nc.vector.tensor_copy(fp_t[:, (PAD + H) * WP:WP * WP],
                                      zeros[:])
                zv = zeros[:].rearrange("p (r c) -> p r c", c=PAD)
                nc.vector.tensor_copy(v[:, PAD:PAD + H, 0:PAD],
                                      zv[:, 0:H, :])
                nc.vector.tensor_copy(v[:, PAD:PAD + H, PAD + W:WP],
                                      zv[:, 0:H, :])
                fpads.append(fp_t)

            # ---- phase 1: conv + g for every block (keeps PE busy while
            # g/ktile/diag for the first block resolve) ----
            blk_state = {}
            for blk in range(SAMPLES_PER_CORE // 2):
                n0, n1 = 2 * blk, 2 * blk + 1
                fp_t = fpads[blk % 2]

                gsums = smalls.tile([128, NCHUNK], F32, tag=f"gsums{blk}")
                for q in range(NSLAB):
                    # x slab tiles per (sample, 128-chan chunk), f32r
                    xts = {}
                    for s, n in enumerate((n0, n1)):
                        for kc in range(2):
                            xt = xpool.tile([128, SLAB], F32R,
                                            tag=f"x{s}{kc}")
                            nc.sync.dma_start(
                                xt[:],
                                x4[n, kc * 128:(kc + 1) * 128,
                                   q * SLAB:(q + 1) * SLAB].bitcast(F32R))
                            xts[(s, kc)] = xt
                    for c in range(SLAB // CHUNK):
                        j = q * (SLAB // CHUNK) + c  # global chunk index
                        # per-sample [64, CHUNK] psum tiles (own banks); the
                        # ACT evacuation shifts s=1 up to partitions 64-127
                        psf = []
                        for s in range(2):
                            ps = ps_conv.tile([64, CHUNK], F32,
                                              tag=f"convps{s}")
                            psf.append(ps)
                            for kc in range(2):
                                nc.tensor.matmul(
                                    ps[:],
                                    wconv_t[:, kc * 64:(kc + 1) * 64],
                                    xts[(s, kc)][:, c * CHUNK:(c + 1) * CHUNK],
                                    start=(kc == 0), stop=(kc == 1))
                        # evacuate with bias; accum_out gives sum for g
                        dst = _fpad_view(fp_t, 8 * j, 8, 0, 0)
                        for s in range(2):
                            sl = slice(64 * s, 64 * s + 64)
                            nc.scalar.activation(
                                dst[sl], psf[s][:], AF.Identity,
                                bias=convb[sl, 0:1],
                                accum_out=gsums[sl, j:j + 1])

                # in-place relu over the interior
                intr = _fpad_view(fp_t, 0, H, 0, 0)
                nc.vector.tensor_scalar_max(intr, intr, 0.0)

                # ---- g, ktile, diag tiles ----
                gpre = smalls.tile([128, 1], F32, tag=f"gpre{blk}")
                nc.vector.tensor_reduce(gpre[:], gsums[:], op=ALU.add,
                                        axis=mybir.AxisListType.X)
                gt = smalls.tile([128, 1], F32, tag=f"g{blk}")
                nc.scalar.activation(gt[:], gpre[:], AF.Relu,
                                     scale=1.0 / PIX)
                ktile = smalls.tile([128, NTAP], F32, tag=f"ktile{blk}")
                nc.vector.scalar_tensor_tensor(ktile[:], aT[:], gt[:, 0:1],
                                               bT[:], op0=ALU.mult,
                                               op1=ALU.add)

                diags = {}
                for t in TENSOR_TAPS:
                    dg = diagp.tile([128, 128], BF16, tag=f"diag{t}")
                    nc.gpsimd.tensor_scalar_mul(dg[:], ident[:],
                                                ktile[:, t:t + 1])
                    diags[t] = dg
                blk_state[blk] = (fp_t, ktile, diags, n0, n1, gt)

            # ---- phase 2: taps + output matmul per block, per slab ----
            for blk in range(SAMPLES_PER_CORE // 2):
                fp_t, ktile, diags, n0, n1, gt = blk_state[blk]
                for q in range(NSLAB):
                    r0 = q * (SLAB // W)      # first output row of slab
                    nr = SLAB // W            # rows per slab (16)

                    # branch 0 on TensorE: diag-matmul accumulation per chunk
                    o1_t = opool.tile([128, SLAB], BF16, tag="o1")
                    for c in range(SLAB // CHUNK):
                        pso = ps_o1.tile([128, CHUNK], F32, tag="o1ps")
                        for i, t in enumerate(TENSOR_TAPS):
                            _, dy, dx, dil = TAPS[t]
                            rhs = _fpad_view(fp_t, r0 + c * (CHUNK // W),
                                             CHUNK // W, dy * dil, dx * dil)
                            nc.tensor.matmul(pso[:], diags[t][:], rhs,
                                             start=(i == 0),
                                             stop=(i == len(TENSOR_TAPS) - 1))
                        nc.scalar.activation(
                            o1_t[:, c * CHUNK:(c + 1) * CHUNK], pso[:],
                            AF.Copy)
                    o1v = o1_t[:].rearrange("p (r c) -> p r c", c=W)
                    for t in DVE_B0_TAPS:
                        _, dy, dx, dil = TAPS[t]
                        src = _fpad_view(fp_t, r0, nr, dy * dil, dx * dil)
                        tmp = opool.tile([128, SLAB], BF16, tag="dvetmp")
                        tv = tmp[:].rearrange("p (r c) -> p r c", c=W)
                        nc.vector.tensor_scalar_mul(tv, src,
                                                    ktile[:, t:t + 1])
                        nc.vector.tensor_tensor(out=o1v, in0=o1v, in1=tv,
                                                op=ALU.add)

                    # branches 1,2 on DVE / GPSIMD
                    acc_tiles = {}  # branch -> (tile, weight_branch)
                    o2_t = opool.tile([128, SLAB], BF16, tag="o2")
                    o3_t = opool.tile([128, SLAB], BF16, tag="o3")
                    dve_by_branch = {1: [], 2: []}
                    for t in DVE_TAPS:
                        dve_by_branch[TAPS[t][0]].append(t)
                    # scalar_tensor_tensor has no fast-mode uops (1x); the
                    # tensor_scalar (4x) + tensor_tensor (2x) bf16 pair is
                    # 0.75 cyc/elem instead.
                    for br, ot in ((1, o2_t), (2, o3_t)):
                        for i, t in enumerate(dve_by_branch[br]):
                            _, dy, dx, dil = TAPS[t]
                            src = _fpad_view(fp_t, r0, nr, dy * dil,
                                             dx * dil)
                            ov = ot[:].rearrange("p (r c) -> p r c", c=W)
                            if i == 0:
                                nc.vector.tensor_scalar_mul(
                                    ov, src, ktile[:, t:t + 1])
                            else:
                                tmp = opool.tile([128, SLAB], BF16,
                                                 tag="dvetmp")
                                tv = tmp[:].rearrange("p (r c) -> p r c",
                                                      c=W)
                                nc.vector.tensor_scalar_mul(
                                    tv, src, ktile[:, t:t + 1])
                                nc.vector.tensor_tensor(
                                    out=ov, in0=ov, in1=tv, op=ALU.add)
                    og_t = None
                    if GPS_TAPS:
                        # GPSIMD has no scalar_tensor_tensor: use
                        # tensor_scalar into tmp + tensor_tensor accumulate.
                        og_t = opool.tile([128, SLAB], BF16, tag="og")
                        ov = og_t[:].rearrange("p (r c) -> p r c", c=W)
                        for i, t in enumerate(GPS_TAPS):
                            _, dy, dx, dil = TAPS[t]
                            src = _fpad_view(fp_t, r0, nr, dy * dil,
                                             dx * dil)
                            if i == 0:
                                nc.gpsimd.tensor_scalar_mul(
                                    ov, src, ktile[:, t:t + 1])
                            else:
                                tmp = opool.tile([128, SLAB], BF16,
                                                 tag="ogtmp")
                                tv = tmp[:].rearrange("p (r c) -> p r c", c=W)
                                nc.gpsimd.tensor_scalar_mul(
                                    tv, src, ktile[:, t:t + 1])
                                nc.gpsimd.tensor_tensor(
                                    out=ov, in0=ov, in1=tv, op=ALU.add)

                    if DEBUG and blk == 0:
                        sl = (q * SLAB, (q + 1) * SLAB)
                        nc.gpsimd.dma_start(dbg_o[0, :, sl[0]:sl[1]],
                                            o1_t[:])
                        nc.gpsimd.dma_start(dbg_o[1, :, sl[0]:sl[1]],
                                            o2_t[:])
                        nc.gpsimd.dma_start(dbg_o[2, :, sl[0]:sl[1]],
                                            o3_t[:])
                        if og_t is not None:
                            nc.gpsimd.dma_start(dbg_o[3, :, sl[0]:sl[1]],
                                                og_t[:])
                        if q == NSLAB - 1:
                            nc.gpsimd.dma_start(dbg_fpad[:],
                                                fp_t[:])
                            nc.gpsimd.dma_start(dbg_g[:, 0:1], gt[:])
                            nc.gpsimd.dma_start(dbg_g[:, 1:1 + NTAP],
                                                ktile[:])

                    # output matmul: pieces (acc tile, branch weight)
                    pieces = [(o1_t, 0), (o2_t, 1), (o3_t, 2)]
                    if og_t is not None:
                        pieces.append((og_t, 2))
                    OUTW = 1024  # output staging tile width
                    osbs = {}
                    for c in range(SLAB // CHUNK):
                        half, cc = divmod(c, OUTW // CHUNK)
                        for mt in range(3):
                            pss = []
                            for s, psp in ((0, ps_out0), (1, ps_out1)):
                                ps = psp.tile([128, CHUNK], F32,
                                              tag=f"outps{s}")
                                pss.append(ps)
                                for ip, (ot, br) in enumerate(pieces):
                                    lhsT = wout_t[64 * s:64 * s + 64,
                                                  br * COUT + mt * 128:
                                                  br * COUT + (mt + 1) * 128]
                                    rhs = ot[64 * s:64 * s + 64,
                                             c * CHUNK:(c + 1) * CHUNK]
                                    nc.tensor.matmul(
                                        ps[:], lhsT, rhs,
                                        start=(ip == 0),
                                        stop=(ip == len(pieces) - 1))
                            for s in range(2):
                                if cc == 0:
                                    osb_tile = outpool.tile(
                                        [128, OUTW], F32, tag=f"osb{mt}_{s}")
                                    osbs[(half, mt, s)] = osb_tile
                                osb = osbs[(half, mt, s)]
                                nc.scalar.activation(
                                    osb[:, cc * CHUNK:(cc + 1) * CHUNK],
                                    pss[s][:], AF.Identity,
                                    bias=biasout[:, mt:mt + 1])
                                if cc == OUTW // CHUNK - 1:
                                    n = (n0, n1)[s]
                                    # issue on the ACT sequencer: FIFO-after
                                    # the producing activation, and keeps the
                                    # SP sequencer free for x loads
                                    nc.scalar.dma_start(
                                        y4[n, mt * 128:(mt + 1) * 128,
                                           q * SLAB + half * OUTW:
                                           q * SLAB + (half + 1) * OUTW],
                                        osb[:])
    nc.compile()
    return nc


def _get_program():
    if "nc" not in _PROGRAM_CACHE:
        _PROGRAM_CACHE["nc"] = _build_program()
    return _PROGRAM_CACHE["nc"]


def kernel(x, conv_w, conv_b, ck_w, ck_b, ck2_w, ck2_b, ckd4_w, ckd4_b,
           kern_w, kern_b, kern2_w, kern2_b, kernd4_w, kernd4_b,
           fuse_w, fuse_b, fc_w, fc_b):
    x = np.asarray(x, dtype=np.float32)
    conv_w = np.asarray(conv_w, dtype=np.float32)
    conv_b = np.asarray(conv_b, dtype=np.float32)
    fuse_w = np.asarray(fuse_w, dtype=np.float32)
    fuse_b = np.asarray(fuse_b, dtype=np.float32)
    fc_w = np.asarray(fc_w, dtype=np.float32)
    fc_b = np.asarray(fc_b, dtype=np.float32)

    NB = x.shape[0]
    assert NB == N_CORES * SAMPLES_PER_CORE

    # ---- host-side weight folding ----
    # tap affine coefficients: k_t = a_t * g + b_t
    a1 = (float(ck_w) * np.asarray(kern_w)).astype(np.float32)        # [25]
    b1 = (float(ck_w) * np.asarray(kern_b) + float(ck_b)).astype(np.float32)
    a2 = (float(ck2_w) * np.asarray(kern2_w)).astype(np.float32)      # [9]
    b2 = (float(ck2_w) * np.asarray(kern2_b) + float(ck2_b)).astype(np.float32)
    a3 = (float(ckd4_w) * np.asarray(kernd4_w)).astype(np.float32)    # [9]
    b3 = (float(ckd4_w) * np.asarray(kernd4_b) + float(ckd4_b)).astype(np.float32)
    a_all = np.concatenate([a1, a2, a3]).astype(np.float32)           # [43]
    b_all = np.concatenate([b1, b2, b3]).astype(np.float32)
    aT = np.broadcast_to(a_all, (128, NTAP)).copy()
    bT = np.broadcast_to(b_all, (128, NTAP)).copy()

    # folded output weights W_i = fc_w[:, 128i:128(i+1)] @ fuse_w  [384, 64]
    import ml_dtypes
    Wi = [fc_w[:, 128 * i:128 * (i + 1)] @ fuse_w for i in range(3)]
    wout = np.zeros((128, 3 * COUT), dtype=np.float32)
    for i in range(3):
        wt = Wi[i].T.astype(np.float32)           # [64, 384]
        wout[0:64, i * COUT:(i + 1) * COUT] = wt
        wout[64:128, i * COUT:(i + 1) * COUT] = wt
    wout = wout.astype(ml_dtypes.bfloat16)
    bias_out = (fc_w @ np.tile(fuse_b, 3) + fc_b).astype(np.float32)  # [384]
    biasout = bias_out.reshape(3, 128).T.copy()   # [128, 3], col mt

    # conv lhsT per 128-channel K-chunk: [128, 2*64]
    wconv = np.concatenate([conv_w[:, 0:128].T, conv_w[:, 128:256].T],
                           axis=1).astype(np.float32)

    convb = np.concatenate([conv_b, conv_b]).reshape(128, 1).astype(np.float32)
    ident = np.eye(128, dtype=np.float32)

    nc = _get_program()
    in_maps = []
    for core in range(N_CORES):
        xs = x[core * SAMPLES_PER_CORE:(core + 1) * SAMPLES_PER_CORE]
        in_maps.append({
            "x4": np.ascontiguousarray(xs.reshape(SAMPLES_PER_CORE, CIN, PIX)),
            "wconv": wconv, "wout": wout, "aT": aT, "bT": bT,
            "ident": ident, "convb": convb, "biasout": biasout,
        })
    res = run_bass_kernel_spmd(nc, in_maps, list(range(N_CORES)))
    out = np.empty((NB, COUT, H, W), dtype=np.float32)
    for core in range(N_CORES):
        out[core * SAMPLES_PER_CORE:(core + 1) * SAMPLES_PER_CORE] = (
            res.results[core]["y4"].reshape(SAMPLES_PER_CORE, COUT, H, W))
    return out



# revision 8
# speedup vs baseline: 1.0382x; 1.0382x over previous
"""Trainium2 Bass kernel for nn_DIDAModuleD4 (dynamic depthwise conv module).

Data-parallel over batch: 32 samples -> 8 cores x 4 samples.
Per core, samples are processed in 2 blocks of 2 samples; each block maps the
2x64=128 (sample, channel) pairs onto the 128 SBUF partitions.

Math (per sample, with host-side weight folding):
  f   = relu(conv_w @ x + conv_b)                       [64, 4096]
  g   = relu(mean_px(conv_w @ x + conv_b))              [64]
  k_t = a_t * g + b_t            (43 taps, a/b host-folded scalars)
  o_i = sum_t k_t * shift_t(f)   (depthwise; 5x5, 3x3 d2, 3x3 d4)
  out = sum_i W_i @ o_i + bias_out                      [384, 4096]
        where W_i = fc_w[:, 128i:128(i+1)] @ fuse_w  (host-folded)

Engine split (cost-model driven: PE matmul = N*0.42ns regardless of K/M,
DVE ts 4x + tt 2x = 0.78ns/elem, Pool ~2x slower than DVE -> no Pool taps):
  - conv1x1: float32r matmuls; one [128, 512] PSUM tile per chunk holds both
    samples (M=64 writes to partition halves); ACT evacuates with bias and
    accum_out for the g means.
  - 5x5 branch (25 taps): diagonal-matmul PSUM accumulation on PE, taps
    iterated OUTER over 2-chunk groups so each diag Ldweights is shared by
    2 matmuls (PE sequencer relief).
  - 3x3 dil2/dil4 branches (18 taps): DVE tensor_scalar (4x) +
    tensor_tensor (2x) on 2048-px half-images; the last accumulate is split
    into two 64-partition adds that write per-sample branch-pair tiles
    X=(o2_s0;o3_s0), Y=(o2_s1;o3_s1) -> K=128 out matmuls.
  - output: per (sample, mt) psum accumulates one K=128 matmul (X/Y against
    branch-packed wout12) and one K=64 matmul (o1 slice against wout1);
    ACT adds bias on evacuation; stores issued from the ACT sequencer.
Datapath dtypes: conv f32r; f_pad/diag/o/wout bf16; psum fp32; I/O fp32.
"""

import sys

if "/opt/trn_rl_repo" not in sys.path:
    sys.path.insert(0, "/opt/trn_rl_repo")

import os
import numpy as np
from contextlib import ExitStack

from concourse import bass, mybir, tile, bacc
from concourse.bass_utils import run_bass_kernel_spmd

F32 = mybir.dt.float32
F32R = mybir.dt.float32r
BF16 = mybir.dt.bfloat16
AF = mybir.ActivationFunctionType
ALU = mybir.AluOpType

N_CORES = 8
SAMPLES_PER_CORE = 4
CM = 64          # reduced channels / groups
CIN = 256
COUT = 384
H = W = 64
PIX = H * W      # 4096
PAD = 4
WP = W + 2 * PAD  # 72
HALF = 2048      # pixels per half-image (32 rows)
NHALF = PIX // HALF          # 2
CHUNK = 512                  # matmul N (one PSUM bank)
NCHUNK = PIX // CHUNK        # 8
CPH = HALF // CHUNK          # chunks per half (4)
GRP = 2                      # chunks per tap-outer group
SLAB = 1024
NSLAB = PIX // SLAB

# taps: (branch, dy, dx, dilation); ktile column order must match aT/bT
TAPS = (
    [(0, dy, dx, 1) for dy in range(-2, 3) for dx in range(-2, 3)]
    + [(1, dy, dx, 2) for dy in range(-1, 2) for dx in range(-1, 2)]
    + [(2, dy, dx, 4) for dy in range(-1, 2) for dx in range(-1, 2)]
)
NTAP = len(TAPS)  # 43

TENSOR_TAPS = [t for t in range(NTAP) if TAPS[t][0] == 0]   # 25, on PE
DVE_B1 = [t for t in range(NTAP) if TAPS[t][0] == 1]        # 9, on DVE
DVE_B2 = [t for t in range(NTAP) if TAPS[t][0] == 2]        # 9, on DVE

_PROGRAM_CACHE = {}


def _fpad_view(fp_t, r0, nrows, off_r, off_c, dtype=None):
    """View of padded-f tile [128, WP*WP] covering output rows [r0, r0+nrows)
    shifted by (off_r, off_c). Returns [128, nrows, 64] AP."""
    v = fp_t[:].rearrange("p (r c) -> p r c", c=WP)
    if dtype is not None:
        v = v.bitcast(dtype)
    return v[:, PAD + r0 + off_r : PAD + r0 + nrows + off_r,
             PAD + off_c : PAD + W + off_c]


def _build_program():
    nc = bacc.Bacc("TRN2", target_bir_lowering=False, debug=False,
                   num_devices=N_CORES)

    x4 = nc.dram_tensor("x4", [SAMPLES_PER_CORE, CIN, PIX], F32,
                        kind="ExternalInput").ap()
    wconv = nc.dram_tensor("wconv", [128, 512], F32,
                           kind="ExternalInput").ap()
    wout12_d = nc.dram_tensor("wout12", [128, 3 * 128], BF16,
                              kind="ExternalInput").ap()
    wout1_d = nc.dram_tensor("wout1", [128, COUT], BF16,
                             kind="ExternalInput").ap()
    aT_d = nc.dram_tensor("aT", [128, NTAP], F32, kind="ExternalInput").ap()
    bT_d = nc.dram_tensor("bT", [128, NTAP], F32, kind="ExternalInput").ap()
    ident_d = nc.dram_tensor("ident", [128, 128], F32,
                             kind="ExternalInput").ap()
    convb_d = nc.dram_tensor("convb", [128, 1], F32, kind="ExternalInput").ap()
    biasout_d = nc.dram_tensor("biasout", [128, 3], F32,
                               kind="ExternalInput").ap()
    y4 = nc.dram_tensor("y4", [SAMPLES_PER_CORE, COUT, PIX], F32,
                        kind="ExternalOutput").ap()

    with tile.TileContext(nc) as tc:
        with ExitStack() as ctx:
            consts = ctx.enter_context(tc.tile_pool(name="consts", bufs=1))
            xpool = ctx.enter_context(tc.tile_pool(name="xp", bufs=2))
            fpool = ctx.enter_context(tc.tile_pool(name="fp", bufs=1))
            opool = ctx.enter_context(tc.tile_pool(name="op", bufs=2))
            o1pool = ctx.enter_context(tc.tile_pool(name="o1p", bufs=2))
            outpool = ctx.enter_context(tc.tile_pool(name="outp", bufs=2))
            smalls = ctx.enter_context(tc.tile_pool(name="sm", bufs=2))
            diagp = ctx.enter_context(tc.tile_pool(name="dg", bufs=2))
            # PSUM: pool A = 2 banks (conv pairs + o1 tap groups),
            #       pool B = 6 banks (out psums per (s, mt)).
            ps_a = ctx.enter_context(
                tc.tile_pool(name="psa", bufs=1, space="PSUM"))
            ps_out = ctx.enter_context(
                tc.tile_pool(name="pso", bufs=1, space="PSUM"))

            # ---- constants (issued on the GPSIMD sequencer) ----
            wconv_t = consts.tile([128, 512], F32R, tag="wconv")
            nc.gpsimd.dma_start(wconv_t[:], wconv[:].bitcast(F32R))
            wout12_t = consts.tile([128, 3 * 128], BF16, tag="wout12")
            nc.gpsimd.dma_start(wout12_t[:], wout12_d[:])
            wout1_t = consts.tile([128, COUT], BF16, tag="wout1")
            nc.gpsimd.dma_start(wout1_t[:], wout1_d[:])
            aT = consts.tile([128, NTAP], F32, tag="aT")
            nc.gpsimd.dma_start(aT[:], aT_d[:])
            bT = consts.tile([128, NTAP], F32, tag="bT")
            nc.gpsimd.dma_start(bT[:], bT_d[:])
            ident = consts.tile([128, 128], F32, tag="ident")
            nc.gpsimd.dma_start(ident[:], ident_d[:])
            convb = consts.tile([128, 1], F32, tag="convb")
            nc.gpsimd.dma_start(convb[:], convb_d[:])
            biasout = consts.tile([128, 3], F32, tag="biasout")
            nc.gpsimd.dma_start(biasout[:], biasout_d[:])

            # persistent padded-f tiles (one per block parity); borders are
            # zeroed once and never rewritten (interior writes only).
            zeros = consts.tile([128, PAD * WP], F32, tag="zeros")
            nc.gpsimd.memset(zeros[:], 0.0)
            fpads = []
            for par in range(2):
                fp_t = fpool.tile([128, WP * WP], BF16, tag=f"fpad{par}")
                v = fp_t[:].rearrange("p (r c) -> p r c", c=WP)
                nc.vector.tensor_copy(fp_t[:, 0:PAD * WP], zeros[:])
                nc.vector.tensor_copy(fp_t[:, (PAD + H) * WP:WP * WP],
                                      zeros[:])
                zv = zeros[:].rearrange("p (r c) -> p r c", c=PAD)
                nc.vector.tensor_copy(v[:, PAD:PAD + H, 0:PAD],
                                      zv[:, 0:H, :])
                nc.vector.tensor_copy(v[:, PAD:PAD + H, PAD + W:WP],
                                      zv[:, 0:H, :])
                fpads.append(fp_t)

            # ---- phase 1: conv + g for every block (keeps PE busy while
            # g/ktile/diag for the first block resolve) ----
            blk_state = {}
            for blk in range(SAMPLES_PER_CORE // 2):
                n0, n1 = 2 * blk, 2 * blk + 1
                fp_t = fpads[blk % 2]

                gsums = smalls.tile([128, NCHUNK], F32, tag=f"gsums{blk}")
                for q in range(NSLAB):
                    # x slab tiles: partitions = (sample pair, 64-chan
                    # group); block-diag wconv contracts both samples in
                    # one K=128 matmul per 64-channel group.
                    xts = {}
                    for kc in range(4):
                        xt = xpool.tile([128, SLAB], F32R, tag=f"x{kc}")
                        nc.sync.dma_start(
                            xt[:],
                            x4[n0:n0 + 2, kc * 64:(kc + 1) * 64,
                               q * SLAB:(q + 1) * SLAB].bitcast(F32R))
                        xts[kc] = xt
                    for c in range(SLAB // CHUNK):
                        j = q * (SLAB // CHUNK) + c  # global chunk index
                        ps = ps_a.tile([128, CHUNK], F32,
                                       tag=f"psa{j % GRP}")
                        for kc in range(4):
                            nc.tensor.matmul(
                                ps[:],
                                wconv_t[:, kc * 128:(kc + 1) * 128],
                                xts[kc][:, c * CHUNK:(c + 1) * CHUNK],
                                start=(kc == 0), stop=(kc == 3))
                        # evacuate with bias; accum_out gives sum for g
                        dst = _fpad_view(fp_t, 8 * j, 8, 0, 0)
                        nc.scalar.activation(
                            dst, ps[:], AF.Identity,
                            bias=convb[:, 0:1],
                            accum_out=gsums[:, j:j + 1])

                # in-place relu over the interior
                intr = _fpad_view(fp_t, 0, H, 0, 0)
                nc.vector.tensor_scalar_max(intr, intr, 0.0)

                # ---- g, ktile, diag tiles ----
                gpre = smalls.tile([128, 1], F32, tag=f"gpre{blk}")
                nc.vector.tensor_reduce(gpre[:], gsums[:], op=ALU.add,
                                        axis=mybir.AxisListType.X)
                gt = smalls.tile([128, 1], F32, tag=f"g{blk}")
                nc.scalar.activation(gt[:], gpre[:], AF.Relu,
                                     scale=1.0 / PIX)
                ktile = smalls.tile([128, NTAP], F32, tag=f"ktile{blk}")
                nc.vector.scalar_tensor_tensor(ktile[:], aT[:], gt[:, 0:1],
                                               bT[:], op0=ALU.mult,
                                               op1=ALU.add)

                diags = {}
                for t in TENSOR_TAPS:
                    dg = diagp.tile([128, 128], BF16, tag=f"diag{t}")
                    nc.gpsimd.tensor_scalar_mul(dg[:], ident[:],
                                                ktile[:, t:t + 1])
                    diags[t] = dg
                blk_state[blk] = (fp_t, ktile, diags, n0, n1)

            # ---- phase 2: taps + output matmul per block, per half-image.
            # Output matmuls for (blk, h) are emitted after the taps of the
            # NEXT (blk, h) so PE never waits on the DVE branch tiles.
            def emit_out(st):
                """Output matmuls + evac + store for one (blk, h)."""
                fp_t, ktile, n0, n1, h, o1_t, Xt, Yt = st
                osbs = {}
                for g2 in range(CPH // GRP):       # 1024-px store groups
                    for mt in range(3):
                        for s in range(2):
                            osb_tile = outpool.tile(
                                [128, GRP * CHUNK], F32, tag=f"osb{mt}_{s}")
                            osbs[(mt, s)] = osb_tile
                    for cc in range(GRP):
                        c = g2 * GRP + cc          # chunk within half
                        csl = slice(c * CHUNK, (c + 1) * CHUNK)
                        pss = {}
                        # K=128 branch-pair matmuls (lhsT shared across s)
                        for mt in range(3):
                            for s, bt in ((0, Xt), (1, Yt)):
                                ps = ps_out.tile([128, CHUNK], F32,
                                                 tag=f"out{s}_{mt}")
                                pss[(s, mt)] = ps
                                nc.tensor.matmul(
                                    ps[:],
                                    wout12_t[:, mt * 128:(mt + 1) * 128],
                                    bt[:, csl], start=True, stop=False)
                        # K=64 o1 matmuls close the accumulation
                        for mt in range(3):
                            for s in range(2):
                                sl = slice(64 * s, 64 * s + 64)
                                nc.tensor.matmul(
                                    pss[(s, mt)][:],
                                    wout1_t[sl, mt * 128:(mt + 1) * 128],
                                    o1_t[sl, csl], start=False, stop=True)
                        for mt in range(3):
                            for s in range(2):
                                nc.scalar.activation(
                                    osbs[(mt, s)][:, cc * CHUNK:
                                                  (cc + 1) * CHUNK],
                                    pss[(s, mt)][:], AF.Identity,
                                    bias=biasout[:, mt:mt + 1])
                    px0 = h * HALF + g2 * GRP * CHUNK
                    for mt in range(3):
                        for s in range(2):
                            n = (n0, n1)[s]
                            nc.scalar.dma_start(
                                y4[n, mt * 128:(mt + 1) * 128,
                                   px0:px0 + GRP * CHUNK],
                                osbs[(mt, s)][:])

            pending = None
            for blk in range(SAMPLES_PER_CORE // 2):
                fp_t, ktile, diags, n0, n1 = blk_state[blk]
                for h in range(NHALF):
                    r0 = h * (HALF // W)       # first output row (32/half)
                    nr = HALF // W             # rows per half (32)

                    # ---- branch 0 (5x5) on PE: tap-outer over 2-chunk
                    # groups so each diag Ldweights serves GRP matmuls ----
                    o1_t = o1pool.tile([128, HALF], BF16, tag="o1")
                    for g2 in range(CPH // GRP):
                        pso = []
                        for i in range(GRP):
                            pso_i = ps_a.tile([128, CHUNK], F32,
                                              tag=f"psa{i}")
                            pso.append(pso_i)
                        for i, t in enumerate(TENSOR_TAPS):
                            _, dy, dx, dil = TAPS[t]
                            last = (i == len(TENSOR_TAPS) - 1)
                            for cc in range(GRP):
                                c = g2 * GRP + cc
                                rhs = _fpad_view(
                                    fp_t, r0 + c * (CHUNK // W),
                                    CHUNK // W, dy * dil, dx * dil)
                                nc.tensor.matmul(pso[cc][:], diags[t][:],
                                                 rhs, start=(i == 0),
                                                 stop=last)
                        for cc in range(GRP):
                            c = g2 * GRP + cc
                            nc.scalar.activation(
                                o1_t[:, c * CHUNK:(c + 1) * CHUNK],
                                pso[cc][:], AF.Copy)

                    # ---- branches 1,2 on DVE -> X=(o2s0;o3s0) Y=(o2s1;o3s1)
                    Xt = opool.tile([128, HALF], BF16, tag="X")
                    Yt = opool.tile([128, HALF], BF16, tag="Y")
                    for bi, btaps in ((0, DVE_B1), (1, DVE_B2)):
                        psl = slice(64 * bi, 64 * bi + 64)
                        acc = opool.tile([128, HALF], BF16, tag=f"acc{bi}")
                        av = acc[:].rearrange("p (r c) -> p r c", c=W)
                        for i, t in enumerate(btaps):
                            _, dy, dx, dil = TAPS[t]
                            src = _fpad_view(fp_t, r0, nr, dy * dil,
                                             dx * dil)
                            if i == 0:
                                nc.vector.tensor_scalar_mul(
                                    av, src, ktile[:, t:t + 1])
                                continue
                            tmp = opool.tile([128, HALF], BF16, tag="tmp")
                            tv = tmp[:].rearrange("p (r c) -> p r c", c=W)
                            nc.vector.tensor_scalar_mul(
                                tv, src, ktile[:, t:t + 1])
                            if i < len(btaps) - 1:
                                nc.vector.tensor_tensor(
                                    out=av, in0=av, in1=tv, op=ALU.add)
                            else:
                                # split final accumulate into per-sample
                                # halves written to the branch-pair tiles
                                nc.vector.tensor_tensor(
                                    out=Xt[psl, :], in0=acc[0:64, :],
                                    in1=tmp[0:64, :], op=ALU.add)
                                nc.vector.tensor_tensor(
                                    out=Yt[psl, :], in0=acc[64:128, :],
                                    in1=tmp[64:128, :], op=ALU.add)

                    if pending is not None:
                        emit_out(pending)
                    pending = (fp_t, ktile, n0, n1, h, o1_t, Xt, Yt)
            emit_out(pending)
    nc.compile()
    return nc


def _get_program():
    if "nc" not in _PROGRAM_CACHE:
        _PROGRAM_CACHE["nc"] = _build_program()
    return _PROGRAM_CACHE["nc"]


def kernel(x, conv_w, conv_b, ck_w, ck_b, ck2_w, ck2_b, ckd4_w, ckd4_b,
           kern_w, kern_b, kern2_w, kern2_b, kernd4_w, kernd4_b,
           fuse_w, fuse_b, fc_w, fc_b):
    x = np.asarray(x, dtype=np.float32)
    conv_w = np.asarray(conv_w, dtype=np.float32)
    conv_b = np.asarray(conv_b, dtype=np.float32)
    fuse_w = np.asarray(fuse_w, dtype=np.float32)
    fuse_b = np.asarray(fuse_b, dtype=np.float32)
    fc_w = np.asarray(fc_w, dtype=np.float32)
    fc_b = np.asarray(fc_b, dtype=np.float32)

    NB = x.shape[0]
    assert NB == N_CORES * SAMPLES_PER_CORE

    # ---- host-side weight folding ----
    # tap affine coefficients: k_t = a_t * g + b_t
    a1 = (float(ck_w) * np.asarray(kern_w)).astype(np.float32)        # [25]
    b1 = (float(ck_w) * np.asarray(kern_b) + float(ck_b)).astype(np.float32)
    a2 = (float(ck2_w) * np.asarray(kern2_w)).astype(np.float32)      # [9]
    b2 = (float(ck2_w) * np.asarray(kern2_b) + float(ck2_b)).astype(np.float32)
    a3 = (float(ckd4_w) * np.asarray(kernd4_w)).astype(np.float32)    # [9]
    b3 = (float(ckd4_w) * np.asarray(kernd4_b) + float(ckd4_b)).astype(np.float32)
    a_all = np.concatenate([a1, a2, a3]).astype(np.float32)           # [43]
    b_all = np.concatenate([b1, b2, b3]).astype(np.float32)
    aT = np.broadcast_to(a_all, (128, NTAP)).copy()
    bT = np.broadcast_to(b_all, (128, NTAP)).copy()

    # folded output weights W_i = fc_w[:, 128i:128(i+1)] @ fuse_w  [384, 64]
    import ml_dtypes
    Wi = [fc_w[:, 128 * i:128 * (i + 1)] @ fuse_w for i in range(3)]
    # wout12: branch-pair lhsT for X/Y (K=128): rows 0-63 = branch1 (o2)
    # channels, rows 64-127 = branch2 (o3); cols = 3 mt tiles of 128.
    wout12 = np.zeros((128, 3 * 128), dtype=np.float32)
    wout12[0:64, :] = Wi[1].T.reshape(64, COUT)
    wout12[64:128, :] = Wi[2].T.reshape(64, COUT)
    wout12 = wout12.astype(ml_dtypes.bfloat16)
    # wout1: o1 lhsT (K=64 slices per sample half)
    wout1 = np.zeros((128, COUT), dtype=np.float32)
    wout1[0:64, :] = Wi[0].T
    wout1[64:128, :] = Wi[0].T
    wout1 = wout1.astype(ml_dtypes.bfloat16)
    bias_out = (fc_w @ np.tile(fuse_b, 3) + fc_b).astype(np.float32)  # [384]
    biasout = bias_out.reshape(3, 128).T.copy()   # [128, 3], col mt

    # conv lhsT: 4 block-diag [128, 128] groups; group kc contracts input
    # chans [64kc, 64kc+64) for both samples at once (partition halves).
    wconv = np.zeros((128, 512), dtype=np.float32)
    for kc in range(4):
        wt = conv_w[:, 64 * kc:64 * (kc + 1)].T    # [64 in, 64 out]
        wconv[0:64, 128 * kc:128 * kc + 64] = wt
        wconv[64:128, 128 * kc + 64:128 * (kc + 1)] = wt

    convb = np.concatenate([conv_b, conv_b]).reshape(128, 1).astype(np.float32)
    ident = np.eye(128, dtype=np.float32)

    nc = _get_program()
    in_maps = []
    for core in range(N_CORES):
        xs = x[core * SAMPLES_PER_CORE:(core + 1) * SAMPLES_PER_CORE]
        in_maps.append({
            "x4": np.ascontiguousarray(xs.reshape(SAMPLES_PER_CORE, CIN, PIX)),
            "wconv": wconv, "wout12": wout12, "wout1": wout1,
            "aT": aT, "bT": bT,
            "ident": ident, "convb": convb, "biasout": biasout,
        })
    res = run_bass_kernel_spmd(nc, in_maps, list(range(N_CORES)))
    out = np.empty((NB, COUT, H, W), dtype=np.float32)
    for core in range(N_CORES):
        out[core * SAMPLES_PER_CORE:(core + 1) * SAMPLES_PER_CORE] = (
            res.results[core]["y4"].reshape(SAMPLES_PER_CORE, COUT, H, W))
    return out


# revision 17
# speedup vs baseline: 1.1634x; 1.1206x over previous
"""Trainium2 Bass kernel for nn_DIDAModuleD4 (dynamic depthwise conv module).

Data-parallel over batch: 32 samples -> 8 cores x 4 samples.
Per core, samples are processed in 2 blocks of 2 samples; each block maps the
2x64=128 (sample, channel) pairs onto the 128 SBUF partitions.

Math (per sample, with host-side weight folding):
  f   = relu(conv_w @ x + conv_b)                       [64, 4096]
  g   = relu(mean_px(conv_w @ x + conv_b))              [64]
  k_t = a_t * g + b_t            (43 taps, a/b host-folded scalars)
  o_i = sum_t k_t * shift_t(f)   (depthwise; 5x5, 3x3 d2, 3x3 d4)
  out = sum_i W_i @ o_i + bias_out                      [384, 4096]
        where W_i = fc_w[:, 128i:128(i+1)] @ fuse_w  (host-folded)

Engine split (cost-model driven: PE matmul = N*0.42ns regardless of K/M,
DVE ts 4x + tt 2x = 0.78ns/elem, Pool ~2x slower than DVE -> no Pool taps):
  - conv1x1: float32r matmuls; one [128, 512] PSUM tile per chunk holds both
    samples (M=64 writes to partition halves); ACT evacuates with bias and
    accum_out for the g means.
  - 5x5 branch (25 taps): diagonal-matmul PSUM accumulation on PE, taps
    iterated OUTER over 2-chunk groups so each diag Ldweights is shared by
    2 matmuls (PE sequencer relief).
  - 3x3 dil2/dil4 branches (18 taps): DVE tensor_scalar (4x) +
    tensor_tensor (2x) on 2048-px half-images; the last accumulate is split
    into two 64-partition adds that write per-sample branch-pair tiles
    X=(o2_s0;o3_s0), Y=(o2_s1;o3_s1) -> K=128 out matmuls.
  - output: per (sample, mt) psum accumulates one K=128 matmul (X/Y against
    branch-packed wout12) and one K=64 matmul (o1 slice against wout1);
    ACT adds bias on evacuation; stores issued from the ACT sequencer.
Datapath dtypes: conv f32r; f_pad/diag/o/wout bf16; psum fp32; I/O fp32.
"""

import sys

if "/opt/trn_rl_repo" not in sys.path:
    sys.path.insert(0, "/opt/trn_rl_repo")

import os
import numpy as np
from contextlib import ExitStack

from concourse import bass, mybir, tile, bacc
from concourse.bass_utils import run_bass_kernel_spmd

F32 = mybir.dt.float32
F32R = mybir.dt.float32r
BF16 = mybir.dt.bfloat16
AF = mybir.ActivationFunctionType
ALU = mybir.AluOpType

N_CORES = 8
SAMPLES_PER_CORE = 4
CM = 64          # reduced channels / groups
CIN = 256
COUT = 384
H = W = 64
PIX = H * W      # 4096
PAD = 4
WP = W + 2 * PAD  # 72
HALF = 2048      # pixels per half-image (32 rows)
NHALF = PIX // HALF          # 2
CHUNK = 512                  # matmul N (one PSUM bank)
NCHUNK = PIX // CHUNK        # 8
CPH = HALF // CHUNK          # chunks per half (4)
GRP = 2                      # chunks per tap-outer group
SLAB = 1024
NSLAB = PIX // SLAB

# taps: (branch, dy, dx, dilation); ktile column order must match aT/bT
TAPS = (
    [(0, dy, dx, 1) for dy in range(-2, 3) for dx in range(-2, 3)]
    + [(1, dy, dx, 2) for dy in range(-1, 2) for dx in range(-1, 2)]
    + [(2, dy, dx, 4) for dy in range(-1, 2) for dx in range(-1, 2)]
)
NTAP = len(TAPS)  # 43

TENSOR_TAPS = [t for t in range(NTAP) if TAPS[t][0] == 0]   # 25, on PE
DVE_B1 = [t for t in range(NTAP) if TAPS[t][0] == 1]        # 9, on DVE
DVE_B2 = [t for t in range(NTAP) if TAPS[t][0] == 2]        # 9, on DVE

_PROGRAM_CACHE = {}


def _fpad_view(fp_t, r0, nrows, off_r, off_c, dtype=None):
    """View of padded-f tile [128, WP*WP] covering output rows [r0, r0+nrows)
    shifted by (off_r, off_c). Returns [128, nrows, 64] AP."""
    v = fp_t[:].rearrange("p (r c) -> p r c", c=WP)
    if dtype is not None:
        v = v.bitcast(dtype)
    return v[:, PAD + r0 + off_r : PAD + r0 + nrows + off_r,
             PAD + off_c : PAD + W + off_c]


def _build_program():
    nc = bacc.Bacc("TRN2", target_bir_lowering=False, debug=False,
                   num_devices=N_CORES)

    x4 = nc.dram_tensor("x4", [SAMPLES_PER_CORE, CIN, PIX], BF16,
                        kind="ExternalInput").ap()
    wconv = nc.dram_tensor("wconv", [128, 512], BF16,
                           kind="ExternalInput").ap()
    wout12_d = nc.dram_tensor("wout12", [128, 3 * 128], BF16,
                              kind="ExternalInput").ap()
    wout1_d = nc.dram_tensor("wout1", [128, COUT], BF16,
                             kind="ExternalInput").ap()
    aT_d = nc.dram_tensor("aT", [128, NTAP], F32, kind="ExternalInput").ap()
    bT_d = nc.dram_tensor("bT", [128, NTAP], F32, kind="ExternalInput").ap()
    ident_d = nc.dram_tensor("ident", [128, 128], F32,
                             kind="ExternalInput").ap()
    convb_d = nc.dram_tensor("convb", [128, 1], F32, kind="ExternalInput").ap()
    biasout_d = nc.dram_tensor("biasout", [128, 3], F32,
                               kind="ExternalInput").ap()
    y4 = nc.dram_tensor("y4", [SAMPLES_PER_CORE, COUT, PIX], F32,
                        kind="ExternalOutput").ap()

    with tile.TileContext(nc) as tc:
        with ExitStack() as ctx:
            consts = ctx.enter_context(tc.tile_pool(name="consts", bufs=1))
            xpool = ctx.enter_context(tc.tile_pool(name="xp", bufs=3))
            fpool = ctx.enter_context(tc.tile_pool(name="fp", bufs=1))
            opool = ctx.enter_context(tc.tile_pool(name="op", bufs=2))
            o1pool = ctx.enter_context(tc.tile_pool(name="o1p", bufs=2))
            outpool = ctx.enter_context(tc.tile_pool(name="outp", bufs=2))
            smalls = ctx.enter_context(tc.tile_pool(name="sm", bufs=2))
            diagp = ctx.enter_context(tc.tile_pool(name="dg", bufs=2))
            # PSUM: pool A = 2 banks (conv pairs + o1 tap groups),
            #       pool B = 6 banks (out psums per (s, mt)).
            ps_a = ctx.enter_context(
                tc.tile_pool(name="psa", bufs=1, space="PSUM"))
            ps_out = ctx.enter_context(
                tc.tile_pool(name="pso", bufs=1, space="PSUM"))

            # ---- constants (issued on the GPSIMD sequencer) ----
            wconv_t = consts.tile([128, 512], BF16, tag="wconv")
            nc.gpsimd.dma_start(wconv_t[:], wconv[:])
            wout12_t = consts.tile([128, 3 * 128], BF16, tag="wout12")
            nc.gpsimd.dma_start(wout12_t[:], wout12_d[:])
            wout1_t = consts.tile([128, COUT], BF16, tag="wout1")
            nc.gpsimd.dma_start(wout1_t[:], wout1_d[:])
            aT = consts.tile([128, NTAP], F32, tag="aT")
            nc.gpsimd.dma_start(aT[:], aT_d[:])
            bT = consts.tile([128, NTAP], F32, tag="bT")
            nc.gpsimd.dma_start(bT[:], bT_d[:])
            ident = consts.tile([128, 128], F32, tag="ident")
            nc.gpsimd.dma_start(ident[:], ident_d[:])
            convb = consts.tile([128, 1], F32, tag="convb")
            nc.gpsimd.dma_start(convb[:], convb_d[:])
            biasout = consts.tile([128, 3], F32, tag="biasout")
            nc.gpsimd.dma_start(biasout[:], biasout_d[:])

            # persistent padded-f tiles (one per block parity); borders are
            # zeroed once and never rewritten (interior writes only).
            zeros = consts.tile([128, PAD * WP], F32, tag="zeros")
            nc.gpsimd.memset(zeros[:], 0.0)
            fpads = []
            for par in range(2):
                fp_t = fpool.tile([128, WP * WP], BF16, tag=f"fpad{par}")
                v = fp_t[:].rearrange("p (r c) -> p r c", c=WP)
                nc.vector.tensor_copy(fp_t[:, 0:PAD * WP], zeros[:])
                nc.vector.tensor_copy(fp_t[:, (PAD + H) * WP:WP * WP],
                                      zeros[:])
                zv = zeros[:].rearrange("p (r c) -> p r c", c=PAD)
                nc.vector.tensor_copy(v[:, PAD:PAD + H, 0:PAD],
                                      zv[:, 0:H, :])
                nc.vector.tensor_copy(v[:, PAD:PAD + H, PAD + W:WP],
                                      zv[:, 0:H, :])
                fpads.append(fp_t)

            # ---- per-block emitters ----
            def emit_conv(blk):
                """conv + g + ktile + diag tiles for one sample pair."""
                n0, n1 = 2 * blk, 2 * blk + 1
                fp_t = fpads[blk % 2]

                gsums = smalls.tile([128, NCHUNK], F32, tag=f"gsums{blk}")
                for q in range(NSLAB):
                    # x slab tiles: partitions = (sample pair, 64-chan
                    # group); block-diag wconv contracts both samples in
                    # one K=128 matmul per 64-channel group.
                    xts = {}
                    for kc in range(4):
                        xt = xpool.tile([128, SLAB], BF16, tag=f"x{kc}")
                        nc.sync.dma_start(
                            xt[:],
                            x4[n0:n0 + 2, kc * 64:(kc + 1) * 64,
                               q * SLAB:(q + 1) * SLAB])
                        xts[kc] = xt
                    for c in range(SLAB // CHUNK):
                        j = q * (SLAB // CHUNK) + c  # global chunk index
                        ps = ps_a.tile([128, CHUNK], F32,
                                       tag=f"psa{j % GRP}")
                        for kc in range(4):
                            nc.tensor.matmul(
                                ps[:],
                                wconv_t[:, kc * 128:(kc + 1) * 128],
                                xts[kc][:, c * CHUNK:(c + 1) * CHUNK],
                                start=(kc == 0), stop=(kc == 3))
                        # evacuate with bias; accum_out gives sum for g
                        dst = _fpad_view(fp_t, 8 * j, 8, 0, 0)
                        nc.scalar.activation(
                            dst, ps[:], AF.Identity,
                            bias=convb[:, 0:1],
                            accum_out=gsums[:, j:j + 1])

                # in-place relu over the interior
                intr = _fpad_view(fp_t, 0, H, 0, 0)
                nc.vector.tensor_scalar_max(intr, intr, 0.0)

                # ---- g, ktile, diag tiles ----
                gpre = smalls.tile([128, 1], F32, tag=f"gpre{blk}")
                nc.vector.tensor_reduce(gpre[:], gsums[:], op=ALU.add,
                                        axis=mybir.AxisListType.X)
                gt = smalls.tile([128, 1], F32, tag=f"g{blk}")
                nc.scalar.activation(gt[:], gpre[:], AF.Relu,
                                     scale=1.0 / PIX)
                ktile = smalls.tile([128, NTAP], F32, tag=f"ktile{blk}")
                nc.vector.scalar_tensor_tensor(ktile[:], aT[:], gt[:, 0:1],
                                               bT[:], op0=ALU.mult,
                                               op1=ALU.add)

                diags = {}
                for t in TENSOR_TAPS:
                    dg = diagp.tile([128, 128], BF16, tag=f"diag{t}")
                    nc.gpsimd.tensor_scalar_mul(dg[:], ident[:],
                                                ktile[:, t:t + 1])
                    diags[t] = dg
                return (fp_t, ktile, diags, n0, n1)

            def emit_out(st):
                """Output matmuls + evac + store for one (blk, h)."""
                fp_t, ktile, n0, n1, h, o1_t, Xt, Yt = st
                osbs = {}
                for g2 in range(CPH // GRP):       # 1024-px store groups
                    for mt in range(3):
                        for s in range(2):
                            osb_tile = outpool.tile(
                                [128, GRP * CHUNK], F32, tag=f"osb{mt}_{s}")
                            osbs[(mt, s)] = osb_tile
                    for cc in range(GRP):
                        c = g2 * GRP + cc          # chunk within half
                        csl = slice(c * CHUNK, (c + 1) * CHUNK)
                        pss = {}
                        # K=128 branch-pair matmuls (lhsT shared across s)
                        for mt in range(3):
                            for s, bt in ((0, Xt), (1, Yt)):
                                ps = ps_out.tile([128, CHUNK], F32,
                                                 tag=f"out{s}_{mt}")
                                pss[(s, mt)] = ps
                                nc.tensor.matmul(
                                    ps[:],
                                    wout12_t[:, mt * 128:(mt + 1) * 128],
                                    bt[:, csl], start=True, stop=False)
                        # K=64 o1 matmuls close the accumulation
                        for mt in range(3):
                            for s in range(2):
                                sl = slice(64 * s, 64 * s + 64)
                                nc.tensor.matmul(
                                    pss[(s, mt)][:],
                                    wout1_t[sl, mt * 128:(mt + 1) * 128],
                                    o1_t[sl, csl], start=False, stop=True)
                        for mt in range(3):
                            for s in range(2):
                                nc.scalar.activation(
                                    osbs[(mt, s)][:, cc * CHUNK:
                                                  (cc + 1) * CHUNK],
                                    pss[(s, mt)][:], AF.Identity,
                                    bias=biasout[:, mt:mt + 1])
                    px0 = h * HALF + g2 * GRP * CHUNK
                    for mt in range(3):
                        for s in range(2):
                            n = (n0, n1)[s]
                            dst = y4[n, mt * 128:(mt + 1) * 128,
                                     px0:px0 + GRP * CHUNK]
                            # split stores between the HWDGE queue (ACT
                            # issue) and the SWDGE path (Pool engine) —
                            # the two run in parallel in the DMA model
                            if g2 == 0:
                                nc.gpsimd.dma_start(dst, osbs[(mt, s)][:])
                            else:
                                nc.scalar.dma_start(dst, osbs[(mt, s)][:])

            def emit_taps(blk_st, h):
                fp_t, ktile, diags, n0, n1 = blk_st
                if True:
                    r0 = h * (HALF // W)       # first output row (32/half)
                    nr = HALF // W             # rows per half (32)

                    # ---- branch 0 (5x5) on PE: tap-outer over 2-chunk
                    # groups so each diag Ldweights serves GRP matmuls ----
                    o1_t = o1pool.tile([128, HALF], BF16, tag="o1")
                    for g2 in range(CPH // GRP):
                        pso = []
                        for i in range(GRP):
                            pso_i = ps_a.tile([128, CHUNK], F32,
                                              tag=f"psa{i}")
                            pso.append(pso_i)
                        for i, t in enumerate(TENSOR_TAPS):
                            _, dy, dx, dil = TAPS[t]
                            last = (i == len(TENSOR_TAPS) - 1)
                            for cc in range(GRP):
                                c = g2 * GRP + cc
                                rhs = _fpad_view(
                                    fp_t, r0 + c * (CHUNK // W),
                                    CHUNK // W, dy * dil, dx * dil)
                                nc.tensor.matmul(pso[cc][:], diags[t][:],
                                                 rhs, start=(i == 0),
                                                 stop=last)
                        for cc in range(GRP):
                            c = g2 * GRP + cc
                            nc.scalar.activation(
                                o1_t[:, c * CHUNK:(c + 1) * CHUNK],
                                pso[cc][:], AF.Copy)

                    # ---- branches 1,2 on DVE -> X=(o2s0;o3s0) Y=(o2s1;o3s1)
                    Xt = opool.tile([128, HALF], BF16, tag="X")
                    Yt = opool.tile([128, HALF], BF16, tag="Y")
                    for bi, btaps in ((0, DVE_B1), (1, DVE_B2)):
                        psl = slice(64 * bi, 64 * bi + 64)
                        acc = opool.tile([128, HALF], BF16, tag=f"acc{bi}")
                        av = acc[:].rearrange("p (r c) -> p r c", c=W)
                        for i, t in enumerate(btaps):
                            _, dy, dx, dil = TAPS[t]
                            src = _fpad_view(fp_t, r0, nr, dy * dil,
                                             dx * dil)
                            if i == 0:
                                nc.vector.tensor_scalar_mul(
                                    av, src, ktile[:, t:t + 1])
                                continue
                            tmp = opool.tile([128, HALF], BF16, tag="tmp")
                            tv = tmp[:].rearrange("p (r c) -> p r c", c=W)
                            nc.vector.tensor_scalar_mul(
                                tv, src, ktile[:, t:t + 1])
                            if i < len(btaps) - 1:
                                nc.vector.tensor_tensor(
                                    out=av, in0=av, in1=tv, op=ALU.add)
                            else:
                                # split final accumulate into per-sample
                                # halves written to the branch-pair tiles
                                nc.vector.tensor_tensor(
                                    out=Xt[psl, :], in0=acc[0:64, :],
                                    in1=tmp[0:64, :], op=ALU.add)
                                nc.vector.tensor_tensor(
                                    out=Yt[psl, :], in0=acc[64:128, :],
                                    in1=tmp[64:128, :], op=ALU.add)

                    return (fp_t, ktile, n0, n1, h, o1_t, Xt, Yt)

            # ---- schedule: conv(b1) is deferred until after the first
            # tap block so its x loads aren't starved by b0's, and each
            # out(blk, h) is emitted after the taps of the NEXT (blk, h)
            # so PE never waits on the DVE branch tiles ----
            st0 = emit_conv(0)
            p00 = emit_taps(st0, 0)
            st1 = emit_conv(1)
            p01 = emit_taps(st0, 1)
            emit_out(p00)
            p10 = emit_taps(st1, 0)
            emit_out(p01)
            p11 = emit_taps(st1, 1)
            emit_out(p10)
            emit_out(p11)
    nc.compile()
    return nc


def _get_program():
    if "nc" not in _PROGRAM_CACHE:
        _PROGRAM_CACHE["nc"] = _build_program()
    return _PROGRAM_CACHE["nc"]


def kernel(x, conv_w, conv_b, ck_w, ck_b, ck2_w, ck2_b, ckd4_w, ckd4_b,
           kern_w, kern_b, kern2_w, kern2_b, kernd4_w, kernd4_b,
           fuse_w, fuse_b, fc_w, fc_b):
    x = np.asarray(x, dtype=np.float32)
    conv_w = np.asarray(conv_w, dtype=np.float32)
    conv_b = np.asarray(conv_b, dtype=np.float32)
    fuse_w = np.asarray(fuse_w, dtype=np.float32)
    fuse_b = np.asarray(fuse_b, dtype=np.float32)
    fc_w = np.asarray(fc_w, dtype=np.float32)
    fc_b = np.asarray(fc_b, dtype=np.float32)

    NB = x.shape[0]
    assert NB == N_CORES * SAMPLES_PER_CORE

    # ---- host-side weight folding ----
    # tap affine coefficients: k_t = a_t * g + b_t
    a1 = (float(ck_w) * np.asarray(kern_w)).astype(np.float32)        # [25]
    b1 = (float(ck_w) * np.asarray(kern_b) + float(ck_b)).astype(np.float32)
    a2 = (float(ck2_w) * np.asarray(kern2_w)).astype(np.float32)      # [9]
    b2 = (float(ck2_w) * np.asarray(kern2_b) + float(ck2_b)).astype(np.float32)
    a3 = (float(ckd4_w) * np.asarray(kernd4_w)).astype(np.float32)    # [9]
    b3 = (float(ckd4_w) * np.asarray(kernd4_b) + float(ckd4_b)).astype(np.float32)
    a_all = np.concatenate([a1, a2, a3]).astype(np.float32)           # [43]
    b_all = np.concatenate([b1, b2, b3]).astype(np.float32)
    aT = np.broadcast_to(a_all, (128, NTAP)).copy()
    bT = np.broadcast_to(b_all, (128, NTAP)).copy()

    # folded output weights W_i = fc_w[:, 128i:128(i+1)] @ fuse_w  [384, 64]
    import ml_dtypes
    Wi = [fc_w[:, 128 * i:128 * (i + 1)] @ fuse_w for i in range(3)]
    # wout12: branch-pair lhsT for X/Y (K=128): rows 0-63 = branch1 (o2)
    # channels, rows 64-127 = branch2 (o3); cols = 3 mt tiles of 128.
    wout12 = np.zeros((128, 3 * 128), dtype=np.float32)
    wout12[0:64, :] = Wi[1].T.reshape(64, COUT)
    wout12[64:128, :] = Wi[2].T.reshape(64, COUT)
    wout12 = wout12.astype(ml_dtypes.bfloat16)
    # wout1: o1 lhsT (K=64 slices per sample half)
    wout1 = np.zeros((128, COUT), dtype=np.float32)
    wout1[0:64, :] = Wi[0].T
    wout1[64:128, :] = Wi[0].T
    wout1 = wout1.astype(ml_dtypes.bfloat16)
    bias_out = (fc_w @ np.tile(fuse_b, 3) + fc_b).astype(np.float32)  # [384]
    biasout = bias_out.reshape(3, 128).T.copy()   # [128, 3], col mt

    # conv lhsT: 4 block-diag [128, 128] groups; group kc contracts input
    # chans [64kc, 64kc+64) for both samples at once (partition halves).
    wconv = np.zeros((128, 512), dtype=np.float32)
    for kc in range(4):
        wt = conv_w[:, 64 * kc:64 * (kc + 1)].T    # [64 in, 64 out]
        wconv[0:64, 128 * kc:128 * kc + 64] = wt
        wconv[64:128, 128 * kc + 64:128 * (kc + 1)] = wt
    wconv = wconv.astype(ml_dtypes.bfloat16)

    convb = np.concatenate([conv_b, conv_b]).reshape(128, 1).astype(np.float32)
    ident = np.eye(128, dtype=np.float32)

    nc = _get_program()
    in_maps = []
    xbf = x.reshape(NB, CIN, PIX).astype(ml_dtypes.bfloat16)
    for core in range(N_CORES):
        xs = xbf[core * SAMPLES_PER_CORE:(core + 1) * SAMPLES_PER_CORE]
        in_maps.append({
            "x4": np.ascontiguousarray(xs),
            "wconv": wconv, "wout12": wout12, "wout1": wout1,
            "aT": aT, "bT": bT,
            "ident": ident, "convb": convb, "biasout": biasout,
        })
    res = run_bass_kernel_spmd(nc, in_maps, list(range(N_CORES)))
    out = np.empty((NB, COUT, H, W), dtype=np.float32)
    for core in range(N_CORES):
        out[core * SAMPLES_PER_CORE:(core + 1) * SAMPLES_PER_CORE] = (
            res.results[core]["y4"].reshape(SAMPLES_PER_CORE, COUT, H, W))
    return out
